# revision 26
# baseline (speedup 1.0000x reference)
"""Axial (frame-local) attention kernel for Trainium2, 8-core data-parallel.

Problem: x[4, 8192, 512] -> qkv proj -> per-(batch, head, frame) attention over
256-token frames (f=32 frames of 256 tokens in an 8192 sequence) -> out proj.

Sharding: pure data-parallel over (batch, half-sequence): core c handles
batch c//2, tokens (c%2)*4096 .. +4096 (16 whole frames). No collectives.

Per-core pipeline (chunks of 512 tokens):
  - load x chunk, PE-transpose into xT [dim, tok] (feature-major)
  - qT,kT = (w_qkv block)^T-matmul: [feat, tok] layout; v natural [tok, feat]
  - per (frame, head): sim^T = k q^T on PE -> exp on ScalarE (no max-subtract;
    logits are O(6) so fp32 exp is safe) -> ov = [v|1]^T p~ on PE produces both
    the unnormalized attention output AND the softmax denominator Z (row 64)
  - normalize: 1/Z via fast DVE reciprocal, GPSIMD partition-broadcast, one
    fused DVE multiply into the transposed output buffer
  - output projection from transposed layout + bias, DMA out
"""

import sys
import types

import numpy as np

import concourse.bass as bass
import concourse.tile as tile
from concourse import bacc, mybir
from concourse.bass import ts
from concourse.bass_utils import run_bass_kernel_spmd
from concourse.masks import make_identity

F32 = mybir.dt.float32
AF = mybir.ActivationFunctionType
ALU = mybir.AluOpType

# Model dims (hardcoded per problem spec)
B, SEQ, D = 4, 8192, 512
HEADS, DH = 8, 64
INNER = HEADS * DH  # 512
SCALE = DH ** -0.5
FRAME = 256  # n_sp = seq // f = 8192 // 32
N_CORES = 8
T = (B * SEQ) // N_CORES  # 4096 tokens per core
CHUNK = 512  # tokens per inner iteration
NCH = T // CHUNK  # 8
FPC = CHUNK // FRAME  # frames per chunk = 2
SPC = CHUNK // 128  # 128-token subtiles per chunk = 4


def _install_ntff_hook():
    """The trimmed container's antenv lacks axon_hooks; inject it so
    run_bass_kernel_spmd(trace=True) can capture NTFF profiles."""
    if "antenv.axon_hooks" in sys.modules:
        return
    try:
        from trn_agent_boot.trn_boot import _ntff_profile_via_ctypes

        hook = _ntff_profile_via_ctypes("/opt/axon/libaxon_pjrt.so")
    except Exception:
        return
    mod = types.ModuleType("antenv.axon_hooks")
    mod._hook = hook
    mod.get_axon_ntff_profile_hook = lambda: mod._hook
    mod.set_axon_ntff_profile_hook = lambda h: setattr(mod, "_hook", h)
    sys.modules["antenv.axon_hooks"] = mod


FEATURES = set()  # debug knobs: "recip_std", "no_norm", "no_attn", "no_gpsimd"


def _build_body(nc, tc, ctx, x_ap, wqkv_ap, wout_ap, bout_ap, out_ap, n_chunks=NCH):
    pconst = ctx.enter_context(tc.tile_pool(name="const", bufs=1))
    px = ctx.enter_context(tc.tile_pool(name="x", bufs=2))
    pxt = ctx.enter_context(tc.tile_pool(name="xt", bufs=8))
    pqk = ctx.enter_context(tc.tile_pool(name="qk", bufs=16))
    pvx = ctx.enter_context(tc.tile_pool(name="vx", bufs=8))
    ppt = ctx.enter_context(tc.tile_pool(name="pt", bufs=4))
    prz = ctx.enter_context(tc.tile_pool(name="rz", bufs=4))
    prb = ctx.enter_context(tc.tile_pool(name="rb", bufs=4))
    pot = ctx.enter_context(tc.tile_pool(name="ot", bufs=8))
    py = ctx.enter_context(tc.tile_pool(name="y", bufs=4))
    pmm = ctx.enter_context(tc.tile_pool(name="mm", bufs=2, space="PSUM"))
    psim = ctx.enter_context(tc.tile_pool(name="sim", bufs=2, space="PSUM"))
    pov = ctx.enter_context(tc.tile_pool(name="ov", bufs=2, space="PSUM"))

    # Constants
    ident = pconst.tile([128, 128], F32, tag="ident")
    make_identity(nc, ident[:])
    w_sb = pconst.tile([128, 4, 3 * INNER], F32, tag="wqkv")
    nc.sync.dma_start(w_sb[:], wqkv_ap.rearrange("(kt p) e -> p kt e", p=128))
    wo_sb = pconst.tile([128, 4, D], F32, tag="wout")
    nc.sync.dma_start(wo_sb[:], wout_ap.rearrange("(kt p) e -> p kt e", p=128))
    b1 = pconst.tile([1, D], F32, tag="b1")
    nc.sync.dma_start(b1[:], bout_ap.rearrange("(a d) -> a d", a=1))
    bb = pconst.tile([128, D], F32, tag="bb")
    if "no_gpsimd" in FEATURES:
        nc.vector.memset(bb[:], 0.0)
    else:
        nc.gpsimd.partition_broadcast(bb[:], b1[:])

    for ci in range(n_chunks):
        tb = ci * CHUNK

        # ---- load x chunk [128, subtile, D] (token-major) ----
        x_t = px.tile([128, SPC, D], F32, tag="x")
        nc.sync.dma_start(
            x_t[:], x_ap[tb : tb + CHUNK, :].rearrange("(t p) d -> p t d", p=128)
        )

        # ---- transpose to xT: 4 tiles [128 dim, CHUNK tok] ----
        xts = []
        for db in range(4):
            xtp = pmm.tile([128, CHUNK], F32, tag="mm")
            for t in range(SPC):
                nc.tensor.transpose(
                    xtp[:, ts(t, 128)], x_t[:, t, ts(db, 128)], ident[:]
                )
            xt = pxt.tile([128, CHUNK], F32, tag="xt")
            nc.any.tensor_copy(xt[:], xtp[:])
            xts.append(xt)

        # ---- qT, kT in [feat, tok] layout: 8 ptiles of 128 feats ----
        qkts = []
        for p in range(8):
            ps = pmm.tile([128, CHUNK], F32, tag="mm")
            for kt in range(4):
                nc.tensor.matmul(
                    ps[:],
                    w_sb[:, kt, ts(p, 128)],
                    xts[kt][:],
                    start=(kt == 0),
                    stop=(kt == 3),
                )
            qs = pqk.tile([128, CHUNK], F32, tag="qk", bufs=12)
            nc.any.tensor_copy(qs[:], ps[:])
            qkts.append(qs)
        # odd heads live at partitions 64-127; matmul operands must sit at
        # base partition 0 (tile_position row 64 faults on this runtime), so
        # shift them down with SBUF->SBUF DMA (DMA is address-based).
        qkos = []
        for p in range(8):
            qo = pqk.tile([64, CHUNK], F32, tag="qko", name=f"qko{ci}_{p}", bufs=10)
            nc.sync.dma_start(qo[:], qkts[p][64:128, :])
            qkos.append(qo)

        # ---- v natural [tok, feat] + ones column -> vext [128, h, 65] ----
        vexts = []
        for t in range(SPC):
            ps = pmm.tile([128, INNER], F32, tag="mm")
            for kt in range(4):
                nc.tensor.matmul(
                    ps[:],
                    xts[kt][:, ts(t, 128)],
                    w_sb[:, kt, 2 * INNER : 3 * INNER],
                    start=(kt == 0),
                    stop=(kt == 3),
                )
            vx = pvx.tile([128, HEADS, DH + 1], F32, tag="vx", bufs=6)
            nc.vector.memset(vx[:, :, DH : DH + 1], 1.0)
            nc.any.tensor_copy(
                vx[:, :, 0:DH], ps[:].rearrange("p (h d) -> p h d", h=HEADS)
            )
            vexts.append(vx)

        if ci == 0 and any(f.startswith("probe:") for f in FEATURES):
            probe = [f for f in FEATURES if f.startswith("probe:")][0][6:]
            if probe == "sim2":
                pd = pmm.tile([128, 256], F32, tag="mm")
                nc.tensor.matmul(
                    pd[:],
                    qkts[4][0:64, 0:128],
                    qkts[0][0:64, 0:256],
                    start=True,
                    stop=True,
                )
                od = py.tile([128, 256], F32, tag="y", bufs=3)
                nc.vector.tensor_copy(od[:], pd[:])
                nc.sync.dma_start(out_ap[0:128, 0:256], od[:])
                pd2 = pmm.tile([128, 256], F32, tag="mm")
                nc.tensor.matmul(
                    pd2[:],
                    qkos[4][:, 0:128],
                    qkos[0][:, 0:256],
                    start=True,
                    stop=True,
                )
                od2 = py.tile([128, 256], F32, tag="y", bufs=3)
                nc.vector.tensor_copy(od2[:], pd2[:])
                nc.sync.dma_start(out_ap[128:256, 0:256], od2[:])
                return
            if probe == "qkt":
                nc.sync.dma_start(out_ap[0:128, :], qkts[0][:])
                nc.sync.dma_start(out_ap[128:256, :], qkts[4][:])
                nc.sync.dma_start(out_ap[256:320, :], qkos[0][:])
                nc.sync.dma_start(out_ap[320:384, :], qkos[4][:])
            elif probe == "vext":
                nc.sync.dma_start(
                    out_ap[0:128, :].rearrange("p (h d) -> p h d", h=HEADS),
                    vexts[0][:, :, 0:DH],
                )
                nc.sync.dma_start(
                    out_ap[128:256, 0:8], vexts[0][:, :, DH]
                )
            elif probe == "xt":
                nc.sync.dma_start(out_ap[0:128, :], xts[0][:])
                nc.sync.dma_start(out_ap[128:256, :], xts[3][:])
            return

        # ---- attention, output written transposed into outT ptiles ----
        # Even heads (rows 0-63 of a ptile) write otls directly; odd heads
        # compute into base-0 tiles (oto) and are DMA-shifted to rows 64-127.
        otls = [
            pot.tile([128, CHUNK], F32, tag="ot", name=f"ot{ci}_{i}") for i in range(4)
        ]
        otos = [
            pot.tile([64, CHUNK], F32, tag="oto", name=f"oto{ci}_{i}") for i in range(4)
        ]
        for fi in range(FPC):
            f0 = fi * FRAME
            for q in range(2):  # head quads
                pts = []
                for jt in range(2):  # key-side 128-token tiles of the frame
                    sim = psim.tile([128, 4 * FRAME], F32, tag="sim")
                    for hh in range(4):
                        h = q * 4 + hh
                        if h % 2 == 0:
                            ck = qkts[4 + h // 2][0:64, :]
                            cq = qkts[h // 2][0:64, :]
                        else:
                            ck = qkos[4 + h // 2][:]
                            cq = qkos[h // 2][:]
                        nc.tensor.matmul(
                            sim[:, ts(hh, FRAME)],
                            ck[:, f0 + jt * 128 : f0 + (jt + 1) * 128],
                            cq[:, f0 : f0 + FRAME],
                            start=True,
                            stop=True,
                        )
                    pt = ppt.tile([128, 4 * FRAME], F32, tag="pt")
                    nc.scalar.activation(pt[:], sim[:], AF.Exp, scale=SCALE)
                    pts.append(pt)
                    if (
                        "probe:pt" in FEATURES
                        and (ci, fi, q, jt) == (0, 0, 0, 0)
                    ):
                        nc.sync.dma_start(out_ap[0:128, :], pt[:, 0:512])
                        nc.sync.dma_start(out_ap[128:256, :], pt[:, 512:1024])
                        sc = py.tile([128, 4 * FRAME], F32, tag="sc", bufs=1)
                        nc.vector.tensor_copy(sc[:], sim[:])
                        nc.sync.dma_start(out_ap[256:384, :], sc[:, 0:512])
                        nc.sync.dma_start(out_ap[384:512, :], sc[:, 512:1024])
                for hh in range(4):
                    h = q * 4 + hh
                    ov = pov.tile([DH + 1, FRAME], F32, tag="ov")
                    for jt in range(2):
                        nc.tensor.matmul(
                            ov[:],
                            vexts[fi * 2 + jt][:, h, :],
                            pts[jt][:, ts(hh, FRAME)],
                            start=(jt == 0),
                            stop=(jt == 1),
                        )
                    dst = otls[h // 2][0:64] if h % 2 == 0 else otos[h // 2][:]
                    if "no_norm" in FEATURES:
                        nc.vector.tensor_copy(dst[:, f0 : f0 + FRAME], ov[0:DH, :])
                        continue
                    rz = prz.tile([1, FRAME], F32, tag="rz")
                    # NB: reciprocal_approx_fast (custom DVE ucode) corrupts
                    # the DVE on this runtime path — use the standard op.
                    nc.vector.reciprocal(rz[:], ov[DH : DH + 1, :])
                    rb = prb.tile([64, FRAME], F32, tag="rb")
                    if "no_bcast" in FEATURES:
                        nc.vector.memset(rb[:], 1.0)
                    else:
                        nc.gpsimd.partition_broadcast(rb[:], rz[:])
                    nc.vector.tensor_mul(
                        dst[:, f0 : f0 + FRAME], ov[0:DH, :], rb[:]
                    )
                    if "probe:ov" in FEATURES and (ci, fi, h) in (
                        (0, 0, 0),
                        (0, 0, 1),
                    ):
                        r = 0 if h == 0 else 256
                        oc = py.tile([DH + 1, FRAME], F32, tag="oc", bufs=2)
                        nc.vector.tensor_copy(oc[:], ov[:])
                        nc.sync.dma_start(out_ap[r : r + DH + 1, 0:FRAME], oc[:])
                        nc.sync.dma_start(
                            out_ap[r + 65 : r + 66, 0:FRAME], rz[:]
                        )
                        nc.sync.dma_start(
                            out_ap[r + 66 : r + 130, 0:FRAME], rb[:]
                        )
                        nc.sync.dma_start(
                            out_ap[r + 130 : r + 194, 0:FRAME],
                            dst[:, f0 : f0 + FRAME],
                        )
        for p in range(4):
            nc.sync.dma_start(otls[p][64:128, :], otos[p][:])

        # ---- output projection + bias ----
        for s in range(SPC):
            ps = pmm.tile([128, D], F32, tag="mm")
            for p in range(4):
                nc.tensor.matmul(
                    ps[:],
                    otls[p][:, ts(s, 128)],
                    wo_sb[:, p, :],
                    start=(p == 0),
                    stop=(p == 3),
                )
            y = py.tile([128, D], F32, tag="y", bufs=3)
            nc.vector.scalar_tensor_tensor(
                y[:], ps[:], 1.0, bb[:], op0=ALU.mult, op1=ALU.add
            )
            if not any(f.startswith("probe:") for f in FEATURES):
                nc.sync.dma_start(out_ap[tb + s * 128 : tb + (s + 1) * 128, :], y[:])


def tb_f(fi):
    return fi * FRAME


_CACHE = {}


def _get_nc(n_chunks=NCH):
    key = ("nc", n_chunks)
    if key in _CACHE:
        return _CACHE[key]
    from contextlib import ExitStack

    nc = bacc.Bacc("TRN2", target_bir_lowering=False, debug=False, num_devices=N_CORES)
    t_tok = n_chunks * CHUNK
    x_ap = nc.dram_tensor("x", [t_tok, D], F32, kind="ExternalInput").ap()
    wqkv_ap = nc.dram_tensor("w_qkv", [D, 3 * INNER], F32, kind="ExternalInput").ap()
    wout_ap = nc.dram_tensor("w_out", [INNER, D], F32, kind="ExternalInput").ap()
    bout_ap = nc.dram_tensor("b_out", [D], F32, kind="ExternalInput").ap()
    out_ap = nc.dram_tensor("out", [t_tok, D], F32, kind="ExternalOutput").ap()
    with tile.TileContext(nc) as tc:
        with ExitStack() as ctx:
            _build_body(
                nc, tc, ctx, x_ap, wqkv_ap, wout_ap, bout_ap, out_ap, n_chunks=n_chunks
            )
    nc.compile()
    _CACHE[key] = nc
    return nc


def _make_in_maps(x, w_qkv, w_out, b_out):
    x = np.ascontiguousarray(np.asarray(x, dtype=np.float32))
    w_qkv = np.ascontiguousarray(np.asarray(w_qkv, dtype=np.float32))
    w_out = np.ascontiguousarray(np.asarray(w_out, dtype=np.float32))
    b_out = np.ascontiguousarray(np.asarray(b_out, dtype=np.float32))
    assert x.shape == (B, SEQ, D), x.shape
    in_maps = []
    for c in range(N_CORES):
        b = c // 2
        t0 = (c % 2) * T
        in_maps.append(
            {
                "x": np.ascontiguousarray(x[b, t0 : t0 + T, :]),
                "w_qkv": w_qkv,
                "w_out": w_out,
                "b_out": b_out,
            }
        )
    return in_maps


def _assemble(results):
    out = np.empty((B, SEQ, D), dtype=np.float32)
    for c in range(N_CORES):
        b = c // 2
        t0 = (c % 2) * T
        out[b, t0 : t0 + T, :] = results[c]["out"]
    return out


def run(x, w_qkv, w_out, b_out, f=32, trace=False):
    assert int(f) == 32, f"kernel hardcoded for f=32, got {f}"
    _install_ntff_hook()
    nc = _get_nc()
    in_maps = _make_in_maps(x, w_qkv, w_out, b_out)
    res = run_bass_kernel_spmd(nc, in_maps, list(range(N_CORES)), trace=trace)
    return _assemble(res.results), res


def kernel(x, w_qkv, w_out, b_out, f=32):
    out, _ = run(x, w_qkv, w_out, b_out, f=f, trace=False)
    return out


# revision 30
# speedup vs baseline: 2.3984x; 2.3984x over previous
"""Axial (frame-local) attention kernel for Trainium2, 8-core data-parallel.

Problem: x[4, 8192, 512] -> qkv proj -> per-(batch, head, frame) attention over
256-token frames (f=32 frames of 256 tokens in an 8192 sequence) -> out proj.

Sharding: pure data-parallel over (batch, half-sequence): core c handles
batch c//2, tokens (c%2)*4096 .. +4096 (16 whole frames). No collectives.

Per-core pipeline (chunks of 512 tokens):
  - load x chunk, PE-transpose into xT [dim, tok] (feature-major)
  - qT,kT = (w_qkv block)^T-matmul in [feat, tok] layout; v natural [tok, feat]
  - per (frame, head): sim^T = k q^T on PE -> exp on ScalarE (no max-subtract;
    logits are O(6) so fp32 exp is safe) -> ov = [v|1]^T p~ on PE produces both
    the unnormalized attention output AND the softmax denominator Z (row 64)
  - normalize: 1/Z = exp(-ln Z) on ScalarE (DVE reciprocal is 8 cyc/elem —
    too slow), GPSIMD partition-broadcast, one DVE multiply
  - output projection from the transposed layout + bias, DMA out

Matmul operands use float32r (single-pass fp32, ~tf32 precision, 2x faster
than fp32's LOW_HIGH two-pass mode). PSUM accumulation stays fp32.
"""

import sys
import types

import numpy as np

import concourse.bass as bass
import concourse.tile as tile
from concourse import bacc, mybir
from concourse.bass import ts
from concourse.bass_utils import run_bass_kernel_spmd
from concourse.masks import make_identity

F32 = mybir.dt.float32
F32R = mybir.dt.float32r
AF = mybir.ActivationFunctionType
ALU = mybir.AluOpType

# Model dims (hardcoded per problem spec)
B, SEQ, D = 4, 8192, 512
HEADS, DH = 8, 64
INNER = HEADS * DH  # 512
SCALE = DH ** -0.5
FRAME = 256  # n_sp = seq // f = 8192 // 32
N_CORES = 8
T = (B * SEQ) // N_CORES  # 4096 tokens per core
CHUNK = 512  # tokens per inner iteration
NCH = T // CHUNK  # 8
FPC = CHUNK // FRAME  # frames per chunk = 2
SPC = CHUNK // 128  # 128-token subtiles per chunk = 4

# matmul operand dtype: F32R (single-pass, ~tf32) or F32 (two-pass, exact)
MM_DT = F32R

FEATURES = set()  # retained for debug scripts


def _install_ntff_hook():
    """The trimmed container's antenv lacks axon_hooks; inject it so
    run_bass_kernel_spmd(trace=True) can capture NTFF profiles."""
    if "antenv.axon_hooks" in sys.modules:
        return
    try:
        from trn_agent_boot.trn_boot import _ntff_profile_via_ctypes

        hook = _ntff_profile_via_ctypes("/opt/axon/libaxon_pjrt.so")
    except Exception:
        return
    mod = types.ModuleType("antenv.axon_hooks")
    mod._hook = hook
    mod.get_axon_ntff_profile_hook = lambda: mod._hook
    mod.set_axon_ntff_profile_hook = lambda h: setattr(mod, "_hook", h)
    sys.modules["antenv.axon_hooks"] = mod


def _build_body(nc, tc, ctx, x_ap, wqkv_ap, wout_ap, bout_ap, out_ap, n_chunks=NCH):
    mm_dt = MM_DT
    pconst = ctx.enter_context(tc.tile_pool(name="const", bufs=1))
    px = ctx.enter_context(tc.tile_pool(name="x", bufs=2))
    pxt = ctx.enter_context(tc.tile_pool(name="xt", bufs=8))
    pqk = ctx.enter_context(tc.tile_pool(name="qk", bufs=16))
    pvx = ctx.enter_context(tc.tile_pool(name="vx", bufs=6))
    ppt = ctx.enter_context(tc.tile_pool(name="pt", bufs=3))
    prz = ctx.enter_context(tc.tile_pool(name="rz", bufs=3))
    prb = ctx.enter_context(tc.tile_pool(name="rb", bufs=3))
    pov = ctx.enter_context(tc.tile_pool(name="ovs", bufs=3))
    pot = ctx.enter_context(tc.tile_pool(name="ot", bufs=6))
    py = ctx.enter_context(tc.tile_pool(name="y", bufs=3))
    pmm = ctx.enter_context(tc.tile_pool(name="mm", bufs=2, space="PSUM"))
    psim = ctx.enter_context(tc.tile_pool(name="sim", bufs=2, space="PSUM"))
    povp = ctx.enter_context(tc.tile_pool(name="ovp", bufs=2, space="PSUM"))

    # Constants
    ident = pconst.tile([128, 128], F32, tag="ident")
    make_identity(nc, ident[:])
    w_sb = pconst.tile([128, 4, 3 * INNER], mm_dt, tag="wqkv")
    nc.sync.dma_start(
        w_sb[:], wqkv_ap.bitcast(mm_dt).rearrange("(kt p) e -> p kt e", p=128)
    )
    wo_sb = pconst.tile([128, 4, D], mm_dt, tag="wout")
    nc.sync.dma_start(
        wo_sb[:], wout_ap.bitcast(mm_dt).rearrange("(kt p) e -> p kt e", p=128)
    )
    b1 = pconst.tile([1, D], F32, tag="b1")
    nc.sync.dma_start(b1[:], bout_ap.rearrange("(a d) -> a d", a=1))
    bb = pconst.tile([128, D], F32, tag="bb")
    nc.gpsimd.partition_broadcast(bb[:], b1[:])

    for ci in range(n_chunks):
        tb = ci * CHUNK

        # ---- load x chunk [128, subtile, D] (token-major) ----
        x_t = px.tile([128, SPC, D], F32, tag="x")
        nc.sync.dma_start(
            x_t[:], x_ap[tb : tb + CHUNK, :].rearrange("(t p) d -> p t d", p=128)
        )

        # ---- transpose to xT: 4 tiles [128 dim, CHUNK tok] ----
        xts = []
        for db in range(4):
            xtp = pmm.tile([128, CHUNK], F32, tag="mm")
            for t in range(SPC):
                nc.tensor.transpose(
                    xtp[:, ts(t, 128)], x_t[:, t, ts(db, 128)], ident[:]
                )
            xt = pxt.tile([128, CHUNK], mm_dt, tag="xt")
            nc.any.tensor_copy(xt[:], xtp[:])
            xts.append(xt)

        # ---- qT, kT in [feat, tok] layout: 8 ptiles of 128 feats ----
        qkts = []
        for p in range(8):
            ps = pmm.tile([128, CHUNK], F32, tag="mm")
            for kt in range(4):
                nc.tensor.matmul(
                    ps[:],
                    w_sb[:, kt, ts(p, 128)],
                    xts[kt][:],
                    start=(kt == 0),
                    stop=(kt == 3),
                )
            qs = pqk.tile([128, CHUNK], mm_dt, tag="qk", bufs=10)
            nc.vector.tensor_copy(qs[:], ps[:])
            qkts.append(qs)
        # odd heads live at partitions 64-127; matmul operands must sit at
        # base partition 0 (tile_position row 64 faults on this runtime), so
        # shift them down with SBUF->SBUF DMA (DMA is address-based).
        qkos = []
        for p in range(8):
            qo = pqk.tile([64, CHUNK], mm_dt, tag="qko", name=f"qko{ci}_{p}", bufs=9)
            nc.sync.dma_start(qo[:], qkts[p][64:128, :])
            qkos.append(qo)

        # ---- v natural [tok, feat] + ones column -> vext [128, h, 65] ----
        vexts = []
        for t in range(SPC):
            ps = pmm.tile([128, INNER], F32, tag="mm")
            for kt in range(4):
                nc.tensor.matmul(
                    ps[:],
                    xts[kt][:, ts(t, 128)],
                    w_sb[:, kt, 2 * INNER : 3 * INNER],
                    start=(kt == 0),
                    stop=(kt == 3),
                )
            vx = pvx.tile([128, HEADS, DH + 1], mm_dt, tag="vx", bufs=5)
            nc.vector.memset(vx[:, :, DH : DH + 1].bitcast(F32), 1.0)
            nc.vector.tensor_copy(
                vx[:, :, 0:DH], ps[:].rearrange("p (h d) -> p h d", h=HEADS)
            )
            vexts.append(vx)

        # ---- attention, output written transposed into outT ptiles ----
        # Even heads (rows 0-63 of a ptile) write otls directly; odd heads
        # compute into base-0 tiles (oto) and are DMA-shifted to rows 64-127.
        otls = [
            pot.tile([128, CHUNK], mm_dt, tag="ot", name=f"ot{ci}_{i}")
            for i in range(4)
        ]
        otos = [
            pot.tile([64, CHUNK], mm_dt, tag="oto", name=f"oto{ci}_{i}")
            for i in range(4)
        ]
        for fi in range(FPC):
            f0 = fi * FRAME
            for q in range(2):  # head quads
                pts = []
                for jt in range(2):  # key-side 128-token tiles of the frame
                    sim = psim.tile([128, 4 * FRAME], F32, tag="sim")
                    for hh in range(4):
                        h = q * 4 + hh
                        if h % 2 == 0:
                            ck = qkts[4 + h // 2][0:64, :]
                            cq = qkts[h // 2][0:64, :]
                        else:
                            ck = qkos[4 + h // 2][:]
                            cq = qkos[h // 2][:]
                        nc.tensor.matmul(
                            sim[:, ts(hh, FRAME)],
                            ck[:, f0 + jt * 128 : f0 + (jt + 1) * 128],
                            cq[:, f0 : f0 + FRAME],
                            start=True,
                            stop=True,
                        )
                    pt = ppt.tile([128, 4 * FRAME], mm_dt, tag="pt")
                    nc.scalar.activation(pt[:], sim[:], AF.Exp, scale=SCALE)
                    pts.append(pt)
                for m2 in range(2):  # head pairs within the quad
                    ovp = povp.tile([DH + 1, 2 * FRAME], F32, tag="ovp")
                    for hp in range(2):
                        h = q * 4 + 2 * m2 + hp
                        hh = 2 * m2 + hp
                        for jt in range(2):
                            nc.tensor.matmul(
                                ovp[:, ts(hp, FRAME)],
                                vexts[fi * 2 + jt][:, h, :],
                                pts[jt][:, ts(hh, FRAME)],
                                start=(jt == 0),
                                stop=(jt == 1),
                            )
                    # softmax denominators for both heads: rz = exp(-ln Z)
                    lnz = prz.tile([1, 2 * FRAME], F32, tag="lnz")
                    nc.scalar.activation(lnz[:], ovp[DH : DH + 1, :], AF.Ln)
                    # unnormalized outputs to SBUF; frees the PSUM tile
                    ovs = pov.tile([DH, 2 * FRAME], F32, tag="ovs")
                    nc.vector.tensor_copy(ovs[:], ovp[0:DH, :])
                    rz = prz.tile([1, 2 * FRAME], F32, tag="rz")
                    nc.scalar.activation(rz[:], lnz[:], AF.Exp, scale=-1.0)
                    rb = prb.tile([DH, 2 * FRAME], F32, tag="rb")
                    nc.gpsimd.partition_broadcast(rb[:], rz[:])
                    for hp in range(2):
                        h = q * 4 + 2 * m2 + hp
                        dst = otls[h // 2][0:DH] if h % 2 == 0 else otos[h // 2][:]
                        nc.vector.tensor_mul(
                            dst[:, f0 : f0 + FRAME],
                            ovs[:, ts(hp, FRAME)],
                            rb[:, ts(hp, FRAME)],
                        )
        for p in range(4):
            nc.sync.dma_start(otls[p][64:128, :], otos[p][:])

        # ---- output projection + bias ----
        for s in range(SPC):
            ps = pmm.tile([128, D], F32, tag="mm")
            for p in range(4):
                nc.tensor.matmul(
                    ps[:],
                    otls[p][:, ts(s, 128)],
                    wo_sb[:, p, :],
                    start=(p == 0),
                    stop=(p == 3),
                )
            y = py.tile([128, D], F32, tag="y", bufs=3)
            nc.vector.scalar_tensor_tensor(
                y[:], ps[:], 1.0, bb[:], op0=ALU.mult, op1=ALU.add
            )
            nc.sync.dma_start(out_ap[tb + s * 128 : tb + (s + 1) * 128, :], y[:])


_CACHE = {}


def _get_nc(n_chunks=NCH):
    key = ("nc", n_chunks, str(MM_DT))
    if key in _CACHE:
        return _CACHE[key]
    from contextlib import ExitStack

    nc = bacc.Bacc("TRN2", target_bir_lowering=False, debug=False, num_devices=N_CORES)
    t_tok = n_chunks * CHUNK
    x_ap = nc.dram_tensor("x", [t_tok, D], F32, kind="ExternalInput").ap()
    wqkv_ap = nc.dram_tensor("w_qkv", [D, 3 * INNER], F32, kind="ExternalInput").ap()
    wout_ap = nc.dram_tensor("w_out", [INNER, D], F32, kind="ExternalInput").ap()
    bout_ap = nc.dram_tensor("b_out", [D], F32, kind="ExternalInput").ap()
    out_ap = nc.dram_tensor("out", [t_tok, D], F32, kind="ExternalOutput").ap()
    with tile.TileContext(nc) as tc:
        with ExitStack() as ctx:
            _build_body(
                nc, tc, ctx, x_ap, wqkv_ap, wout_ap, bout_ap, out_ap, n_chunks=n_chunks
            )
    nc.compile()
    _CACHE[key] = nc
    return nc


def _make_in_maps(x, w_qkv, w_out, b_out):
    x = np.ascontiguousarray(np.asarray(x, dtype=np.float32))
    w_qkv = np.ascontiguousarray(np.asarray(w_qkv, dtype=np.float32))
    w_out = np.ascontiguousarray(np.asarray(w_out, dtype=np.float32))
    b_out = np.ascontiguousarray(np.asarray(b_out, dtype=np.float32))
    assert x.shape == (B, SEQ, D), x.shape
    in_maps = []
    for c in range(N_CORES):
        b = c // 2
        t0 = (c % 2) * T
        in_maps.append(
            {
                "x": np.ascontiguousarray(x[b, t0 : t0 + T, :]),
                "w_qkv": w_qkv,
                "w_out": w_out,
                "b_out": b_out,
            }
        )
    return in_maps


def _assemble(results):
    out = np.empty((B, SEQ, D), dtype=np.float32)
    for c in range(N_CORES):
        b = c // 2
        t0 = (c % 2) * T
        out[b, t0 : t0 + T, :] = results[c]["out"]
    return out


def run(x, w_qkv, w_out, b_out, f=32, trace=False):
    assert int(f) == 32, f"kernel hardcoded for f=32, got {f}"
    _install_ntff_hook()
    nc = _get_nc()
    in_maps = _make_in_maps(x, w_qkv, w_out, b_out)
    res = run_bass_kernel_spmd(nc, in_maps, list(range(N_CORES)), trace=trace)
    return _assemble(res.results), res


def kernel(x, w_qkv, w_out, b_out, f=32):
    out, _ = run(x, w_qkv, w_out, b_out, f=f, trace=False)
    return out


# revision 35
# speedup vs baseline: 2.8333x; 1.1813x over previous
"""Axial (frame-local) attention kernel for Trainium2, 8-core data-parallel.

Problem: x[4, 8192, 512] -> qkv proj -> per-(batch, head, frame) attention over
256-token frames (f=32 frames of 256 tokens in an 8192 sequence) -> out proj.

Sharding: pure data-parallel over (batch, half-sequence): core c handles
batch c//2, tokens (c%2)*4096 .. +4096 (16 whole frames). No collectives.

Per-core pipeline (chunks of 512 tokens):
  - load x chunk, PE-transpose into xT [dim, tok] (feature-major)
  - qT,kT = (w_qkv block)^T-matmul in [feat, tok] layout; v natural [tok, feat]
  - per (frame, head): sim^T = k q^T on PE -> exp on ScalarE (no max-subtract;
    logits are O(6) so fp32 exp is safe) -> ov = [v|1]^T p~ on PE produces both
    the unnormalized attention output AND the softmax denominator Z (row 64)
  - normalize: 1/Z = exp(-ln Z) on ScalarE (DVE reciprocal is 8 cyc/elem —
    too slow), GPSIMD partition-broadcast, one DVE multiply
  - output projection from the transposed layout + bias, DMA out

Matmul operands use float32r (single-pass fp32, ~tf32 precision, 2x faster
than fp32's LOW_HIGH two-pass mode). PSUM accumulation stays fp32.
"""

import sys
import types

import numpy as np

import concourse.bass as bass
import concourse.tile as tile
from concourse import bacc, mybir
from concourse.bass import ts
from concourse.bass_utils import run_bass_kernel_spmd
from concourse.masks import make_identity

F32 = mybir.dt.float32
F32R = mybir.dt.float32r
AF = mybir.ActivationFunctionType
ALU = mybir.AluOpType

# Model dims (hardcoded per problem spec)
B, SEQ, D = 4, 8192, 512
HEADS, DH = 8, 64
INNER = HEADS * DH  # 512
SCALE = DH ** -0.5
FRAME = 256  # n_sp = seq // f = 8192 // 32
N_CORES = 8
T = (B * SEQ) // N_CORES  # 4096 tokens per core
CHUNK = 512  # tokens per inner iteration
NCH = T // CHUNK  # 8
FPC = CHUNK // FRAME  # frames per chunk = 2
SPC = CHUNK // 128  # 128-token subtiles per chunk = 4

# matmul operand dtype: F32R (single-pass, ~tf32) or F32 (two-pass, exact)
MM_DT = F32R

FEATURES = set()  # retained for debug scripts


def _install_ntff_hook():
    """The trimmed container's antenv lacks axon_hooks; inject it so
    run_bass_kernel_spmd(trace=True) can capture NTFF profiles."""
    if "antenv.axon_hooks" in sys.modules:
        return
    try:
        from trn_agent_boot.trn_boot import _ntff_profile_via_ctypes

        hook = _ntff_profile_via_ctypes("/opt/axon/libaxon_pjrt.so")
    except Exception:
        return
    mod = types.ModuleType("antenv.axon_hooks")
    mod._hook = hook
    mod.get_axon_ntff_profile_hook = lambda: mod._hook
    mod.set_axon_ntff_profile_hook = lambda h: setattr(mod, "_hook", h)
    sys.modules["antenv.axon_hooks"] = mod


def _pin_act_tables():
    """Exp and Ln both live in the natural_log_exp_and_others table set, but
    the table-load chooser maps each function to the first set containing it,
    so alternating Exp/Ln activations reload tables (~1.3us each) every head
    pair. Restrict Exp/Ln to the combined set in the cached table map so one
    load covers the whole kernel."""
    from concourse.hw_specs import get_activation_tables

    tabs = get_activation_tables(_pin_act_tables.arch)
    keep = "natural_log_exp_and_others"
    if keep not in tabs:
        return
    for name, fns in tabs.items():
        if name != keep:
            fns.discard(AF.Exp)
            fns.discard(AF.Ln)


def _build_body(nc, tc, ctx, x_ap, wqkv_ap, wout_ap, bout_ap, out_ap, n_chunks=NCH):
    mm_dt = MM_DT
    pconst = ctx.enter_context(tc.tile_pool(name="const", bufs=1))
    px = ctx.enter_context(tc.tile_pool(name="x", bufs=2))
    pxt = ctx.enter_context(tc.tile_pool(name="xt", bufs=8))
    pqk = ctx.enter_context(tc.tile_pool(name="qk", bufs=16))
    pvx = ctx.enter_context(tc.tile_pool(name="vx", bufs=6))
    ppt = ctx.enter_context(tc.tile_pool(name="pt", bufs=4))
    prz = ctx.enter_context(tc.tile_pool(name="rz", bufs=3))
    prb = ctx.enter_context(tc.tile_pool(name="rb", bufs=3))
    pov = ctx.enter_context(tc.tile_pool(name="ovs", bufs=3))
    pot = ctx.enter_context(tc.tile_pool(name="ot", bufs=6))
    py = ctx.enter_context(tc.tile_pool(name="y", bufs=3))
    pmm = ctx.enter_context(tc.tile_pool(name="mm", bufs=2, space="PSUM"))
    psim = ctx.enter_context(tc.tile_pool(name="sim", bufs=2, space="PSUM"))
    povp = ctx.enter_context(tc.tile_pool(name="ovp", bufs=2, space="PSUM"))

    # Constants
    ident = pconst.tile([128, 128], F32, tag="ident")
    make_identity(nc, ident[:])
    w_sb = pconst.tile([128, 4, 3 * INNER], mm_dt, tag="wqkv")
    nc.sync.dma_start(
        w_sb[:], wqkv_ap.bitcast(mm_dt).rearrange("(kt p) e -> p kt e", p=128)
    )
    wo_sb = pconst.tile([128, 4, D], mm_dt, tag="wout")
    nc.sync.dma_start(
        wo_sb[:], wout_ap.bitcast(mm_dt).rearrange("(kt p) e -> p kt e", p=128)
    )
    b1 = pconst.tile([1, D], F32, tag="b1")
    nc.sync.dma_start(b1[:], bout_ap.rearrange("(a d) -> a d", a=1))
    bb = pconst.tile([128, D], F32, tag="bb")
    nc.gpsimd.partition_broadcast(bb[:], b1[:])

    for ci in range(n_chunks):
        tb = ci * CHUNK

        # ---- load x chunk [128, subtile, D] (token-major) ----
        x_t = px.tile([128, SPC, D], F32, tag="x")
        nc.sync.dma_start(
            x_t[:], x_ap[tb : tb + CHUNK, :].rearrange("(t p) d -> p t d", p=128)
        )

        # ---- transpose to xT: 4 tiles [128 dim, CHUNK tok] ----
        xts = []
        for db in range(4):
            xtp = pmm.tile([128, CHUNK], F32, tag="mm")
            for t in range(SPC):
                nc.tensor.transpose(
                    xtp[:, ts(t, 128)], x_t[:, t, ts(db, 128)], ident[:]
                )
            xt = pxt.tile([128, CHUNK], mm_dt, tag="xt")
            nc.any.tensor_copy(xt[:], xtp[:])
            xts.append(xt)

        # ---- qT, kT in [feat, tok] layout: 8 ptiles of 128 feats ----
        qkts = []
        for p in range(8):
            ps = pmm.tile([128, CHUNK], F32, tag="mm")
            for kt in range(4):
                nc.tensor.matmul(
                    ps[:],
                    w_sb[:, kt, ts(p, 128)],
                    xts[kt][:],
                    start=(kt == 0),
                    stop=(kt == 3),
                )
            qs = pqk.tile([128, CHUNK], mm_dt, tag="qk", bufs=10)
            nc.vector.tensor_copy(qs[:], ps[:])
            qkts.append(qs)
        # odd heads live at partitions 64-127; matmul operands must sit at
        # base partition 0 (tile_position row 64 faults on this runtime), so
        # shift them down with SBUF->SBUF DMA (DMA is address-based).
        qkos = []
        for p in range(8):
            qo = pqk.tile([64, CHUNK], mm_dt, tag="qko", name=f"qko{ci}_{p}", bufs=9)
            nc.sync.dma_start(qo[:], qkts[p][64:128, :])
            qkos.append(qo)

        # ---- v natural [tok, feat] + ones column -> vext [128, h, 65] ----
        vexts = []
        for t in range(SPC):
            ps = pmm.tile([128, INNER], F32, tag="mm")
            for kt in range(4):
                nc.tensor.matmul(
                    ps[:],
                    xts[kt][:, ts(t, 128)],
                    w_sb[:, kt, 2 * INNER : 3 * INNER],
                    start=(kt == 0),
                    stop=(kt == 3),
                )
            vx = pvx.tile([128, HEADS, DH + 1], mm_dt, tag="vx", bufs=5)
            nc.vector.memset(vx[:, :, DH : DH + 1].bitcast(F32), 1.0)
            nc.vector.tensor_copy(
                vx[:, :, 0:DH], ps[:].rearrange("p (h d) -> p h d", h=HEADS)
            )
            vexts.append(vx)

        # ---- attention, output written transposed into outT ptiles ----
        # Even heads (rows 0-63 of a ptile) write otls directly; odd heads
        # compute into base-0 tiles (oto) and are DMA-shifted to rows 64-127.
        otls = [
            pot.tile([128, CHUNK], mm_dt, tag="ot", name=f"ot{ci}_{i}")
            for i in range(4)
        ]
        otos = [
            pot.tile([64, CHUNK], mm_dt, tag="oto", name=f"oto{ci}_{i}")
            for i in range(4)
        ]
        for fi in range(FPC):
            f0 = fi * FRAME
            for q in range(2):  # head quads
                pts = []
                for jt in range(2):  # key-side 128-token tiles of the frame
                    sim = psim.tile([128, 4 * FRAME], F32, tag="sim")
                    for hh in range(4):
                        h = q * 4 + hh
                        if h % 2 == 0:
                            ck = qkts[4 + h // 2][0:64, :]
                            cq = qkts[h // 2][0:64, :]
                        else:
                            ck = qkos[4 + h // 2][:]
                            cq = qkos[h // 2][:]
                        nc.tensor.matmul(
                            sim[:, ts(hh, FRAME)],
                            ck[:, f0 + jt * 128 : f0 + (jt + 1) * 128],
                            cq[:, f0 : f0 + FRAME],
                            start=True,
                            stop=True,
                        )
                    pt = ppt.tile([128, 4 * FRAME], mm_dt, tag="pt")
                    nc.scalar.activation(pt[:], sim[:], AF.Exp, scale=SCALE)
                    pts.append(pt)
                for m2 in range(2):  # head pairs within the quad
                    ovp = povp.tile([DH + 1, 2 * FRAME], F32, tag="ovp")
                    for hp in range(2):
                        h = q * 4 + 2 * m2 + hp
                        hh = 2 * m2 + hp
                        for jt in range(2):
                            nc.tensor.matmul(
                                ovp[:, ts(hp, FRAME)],
                                vexts[fi * 2 + jt][:, h, :],
                                pts[jt][:, ts(hh, FRAME)],
                                start=(jt == 0),
                                stop=(jt == 1),
                            )
                    # softmax denominators for both heads: rz = exp(-ln Z)
                    lnz = prz.tile([1, 2 * FRAME], F32, tag="lnz")
                    nc.scalar.activation(lnz[:], ovp[DH : DH + 1, :], AF.Ln)
                    # unnormalized outputs to SBUF; frees the PSUM tile
                    ovs = pov.tile([DH, 2 * FRAME], F32, tag="ovs")
                    nc.vector.tensor_copy(ovs[:], ovp[0:DH, :])
                    rz = prz.tile([1, 2 * FRAME], F32, tag="rz")
                    nc.scalar.activation(rz[:], lnz[:], AF.Exp, scale=-1.0)
                    rb = prb.tile([DH, 2 * FRAME], F32, tag="rb")
                    nc.gpsimd.partition_broadcast(rb[:], rz[:])
                    for hp in range(2):
                        h = q * 4 + 2 * m2 + hp
                        dst = otls[h // 2][0:DH] if h % 2 == 0 else otos[h // 2][:]
                        nc.vector.tensor_mul(
                            dst[:, f0 : f0 + FRAME],
                            ovs[:, ts(hp, FRAME)],
                            rb[:, ts(hp, FRAME)],
                        )
                    if fi == FPC - 1:
                        # odd head of this pair is complete: shift its rows
                        # into the ptile now so proj isn't gated on one big
                        # end-of-chunk DMA
                        p = q * 2 + m2
                        nc.sync.dma_start(otls[p][64:128, :], otos[p][:])

        # ---- output projection + bias ----
        for s in range(SPC):
            ps = pmm.tile([128, D], F32, tag="mm")
            for p in range(4):
                nc.tensor.matmul(
                    ps[:],
                    otls[p][:, ts(s, 128)],
                    wo_sb[:, p, :],
                    start=(p == 0),
                    stop=(p == 3),
                )
            y = py.tile([128, D], F32, tag="y", bufs=3)
            nc.vector.scalar_tensor_tensor(
                y[:], ps[:], 1.0, bb[:], op0=ALU.mult, op1=ALU.add
            )
            nc.sync.dma_start(out_ap[tb + s * 128 : tb + (s + 1) * 128, :], y[:])


_CACHE = {}


def _get_nc(n_chunks=NCH):
    key = ("nc", n_chunks, str(MM_DT))
    if key in _CACHE:
        return _CACHE[key]
    from contextlib import ExitStack

    nc = bacc.Bacc("TRN2", target_bir_lowering=False, debug=False, num_devices=N_CORES)
    _pin_act_tables.arch = nc.m.arch
    _pin_act_tables()
    t_tok = n_chunks * CHUNK
    x_ap = nc.dram_tensor("x", [t_tok, D], F32, kind="ExternalInput").ap()
    wqkv_ap = nc.dram_tensor("w_qkv", [D, 3 * INNER], F32, kind="ExternalInput").ap()
    wout_ap = nc.dram_tensor("w_out", [INNER, D], F32, kind="ExternalInput").ap()
    bout_ap = nc.dram_tensor("b_out", [D], F32, kind="ExternalInput").ap()
    out_ap = nc.dram_tensor("out", [t_tok, D], F32, kind="ExternalOutput").ap()
    with tile.TileContext(nc) as tc:
        with ExitStack() as ctx:
            _build_body(
                nc, tc, ctx, x_ap, wqkv_ap, wout_ap, bout_ap, out_ap, n_chunks=n_chunks
            )
    nc.compile()
    _CACHE[key] = nc
    return nc


def _make_in_maps(x, w_qkv, w_out, b_out):
    x = np.ascontiguousarray(np.asarray(x, dtype=np.float32))
    w_qkv = np.ascontiguousarray(np.asarray(w_qkv, dtype=np.float32))
    w_out = np.ascontiguousarray(np.asarray(w_out, dtype=np.float32))
    b_out = np.ascontiguousarray(np.asarray(b_out, dtype=np.float32))
    assert x.shape == (B, SEQ, D), x.shape
    in_maps = []
    for c in range(N_CORES):
        b = c // 2
        t0 = (c % 2) * T
        in_maps.append(
            {
                "x": np.ascontiguousarray(x[b, t0 : t0 + T, :]),
                "w_qkv": w_qkv,
                "w_out": w_out,
                "b_out": b_out,
            }
        )
    return in_maps


def _assemble(results):
    out = np.empty((B, SEQ, D), dtype=np.float32)
    for c in range(N_CORES):
        b = c // 2
        t0 = (c % 2) * T
        out[b, t0 : t0 + T, :] = results[c]["out"]
    return out


def run(x, w_qkv, w_out, b_out, f=32, trace=False):
    assert int(f) == 32, f"kernel hardcoded for f=32, got {f}"
    _install_ntff_hook()
    nc = _get_nc()
    in_maps = _make_in_maps(x, w_qkv, w_out, b_out)
    res = run_bass_kernel_spmd(nc, in_maps, list(range(N_CORES)), trace=trace)
    return _assemble(res.results), res


def kernel(x, w_qkv, w_out, b_out, f=32):
    out, _ = run(x, w_qkv, w_out, b_out, f=f, trace=False)
    return out


# revision 36
# speedup vs baseline: 3.9265x; 1.3859x over previous
"""Axial (frame-local) attention kernel for Trainium2, 8-core data-parallel.

Problem: x[4, 8192, 512] -> qkv proj -> per-(batch, head, frame) attention over
256-token frames (f=32 frames of 256 tokens in an 8192 sequence) -> out proj.

Sharding: pure data-parallel over (batch, half-sequence): core c handles
batch c//2, tokens (c%2)*4096 .. +4096 (16 whole frames). No collectives.

Per-core pipeline (chunks of 512 tokens):
  - load x chunk, PE-transpose into xT [dim, tok] (feature-major)
  - qT,kT = (w_qkv block)^T-matmul in [feat, tok] layout; v natural [tok, feat]
  - per (frame, head): sim^T = k q^T on PE -> exp on ScalarE (no max-subtract;
    logits are O(6) so fp32 exp is safe) -> ov = [v|1]^T p~ on PE produces both
    the unnormalized attention output AND the softmax denominator Z (row 64)
  - normalize: 1/Z = exp(-ln Z) on ScalarE (DVE reciprocal is 8 cyc/elem —
    too slow), GPSIMD partition-broadcast, one DVE multiply
  - output projection from the transposed layout + bias, DMA out

Matmul operands use float32r (single-pass fp32, ~tf32 precision, 2x faster
than fp32's LOW_HIGH two-pass mode). PSUM accumulation stays fp32.
"""

import sys
import types

import numpy as np

import concourse.bass as bass
import concourse.tile as tile
from concourse import bacc, mybir
from concourse.bass import ts
from concourse.bass_utils import run_bass_kernel_spmd
from concourse.masks import make_identity

F32 = mybir.dt.float32
F32R = mybir.dt.float32r
AF = mybir.ActivationFunctionType
ALU = mybir.AluOpType

# Model dims (hardcoded per problem spec)
B, SEQ, D = 4, 8192, 512
HEADS, DH = 8, 64
INNER = HEADS * DH  # 512
SCALE = DH ** -0.5
FRAME = 256  # n_sp = seq // f = 8192 // 32
N_CORES = 8
T = (B * SEQ) // N_CORES  # 4096 tokens per core
CHUNK = 512  # tokens per inner iteration
NCH = T // CHUNK  # 8
FPC = CHUNK // FRAME  # frames per chunk = 2
SPC = CHUNK // 128  # 128-token subtiles per chunk = 4

# matmul operand dtype: F32R (single-pass, ~tf32) or F32 (two-pass, exact)
MM_DT = F32R

FEATURES = set()  # retained for debug scripts


def _install_ntff_hook():
    """The trimmed container's antenv lacks axon_hooks; inject it so
    run_bass_kernel_spmd(trace=True) can capture NTFF profiles."""
    if "antenv.axon_hooks" in sys.modules:
        return
    try:
        from trn_agent_boot.trn_boot import _ntff_profile_via_ctypes

        hook = _ntff_profile_via_ctypes("/opt/axon/libaxon_pjrt.so")
    except Exception:
        return
    mod = types.ModuleType("antenv.axon_hooks")
    mod._hook = hook
    mod.get_axon_ntff_profile_hook = lambda: mod._hook
    mod.set_axon_ntff_profile_hook = lambda h: setattr(mod, "_hook", h)
    sys.modules["antenv.axon_hooks"] = mod


def _pin_act_tables():
    """Exp and Ln both live in the natural_log_exp_and_others table set, but
    the table-load chooser maps each function to the first set containing it,
    so alternating Exp/Ln activations reload tables (~1.3us each) every head
    pair. Restrict Exp/Ln to the combined set in the cached table map so one
    load covers the whole kernel."""
    from concourse.hw_specs import get_activation_tables

    tabs = get_activation_tables(_pin_act_tables.arch)
    keep = "natural_log_exp_and_others"
    if keep not in tabs:
        return
    for name, fns in tabs.items():
        if name != keep:
            fns.discard(AF.Exp)
            fns.discard(AF.Ln)


def _build_body(nc, tc, ctx, x_ap, wqkv_ap, wout_ap, bout_ap, out_ap, n_chunks=NCH):
    mm_dt = MM_DT
    pconst = ctx.enter_context(tc.tile_pool(name="const", bufs=1))
    px = ctx.enter_context(tc.tile_pool(name="x", bufs=2))
    pxt = ctx.enter_context(tc.tile_pool(name="xt", bufs=8))
    pqk = ctx.enter_context(tc.tile_pool(name="qk", bufs=16))
    pvx = ctx.enter_context(tc.tile_pool(name="vx", bufs=6))
    ppt = ctx.enter_context(tc.tile_pool(name="pt", bufs=4))
    prz = ctx.enter_context(tc.tile_pool(name="rz", bufs=3))
    prb = ctx.enter_context(tc.tile_pool(name="rb", bufs=3))
    pov = ctx.enter_context(tc.tile_pool(name="ovs", bufs=3))
    pot = ctx.enter_context(tc.tile_pool(name="ot", bufs=6))
    py = ctx.enter_context(tc.tile_pool(name="y", bufs=3))
    pmm = ctx.enter_context(tc.tile_pool(name="mm", bufs=2, space="PSUM"))
    psim = ctx.enter_context(tc.tile_pool(name="sim", bufs=2, space="PSUM"))
    povp = ctx.enter_context(tc.tile_pool(name="ovp", bufs=2, space="PSUM"))

    # Constants
    ident = pconst.tile([128, 128], F32, tag="ident")
    make_identity(nc, ident[:])
    w_kts = []
    for kt in range(4):
        wk = pconst.tile([128, 3 * INNER], mm_dt, tag=f"wqkv{kt}", name=f"wqkv{kt}")
        nc.sync.dma_start(
            wk[:], wqkv_ap.bitcast(mm_dt)[kt * 128 : (kt + 1) * 128, :]
        )
        w_kts.append(wk)
    wo_sb = pconst.tile([128, 4, D], mm_dt, tag="wout")
    nc.sync.dma_start(
        wo_sb[:], wout_ap.bitcast(mm_dt).rearrange("(kt p) e -> p kt e", p=128)
    )
    b1 = pconst.tile([1, D], F32, tag="b1")
    nc.sync.dma_start(b1[:], bout_ap.rearrange("(a d) -> a d", a=1))
    bb = pconst.tile([128, D], F32, tag="bb")
    nc.gpsimd.partition_broadcast(bb[:], b1[:])

    def front(ci):
        tb = ci * CHUNK

        # ---- load x chunk [128, subtile, D] (token-major) ----
        x_t = px.tile([128, SPC, D], F32, tag="x")
        nc.sync.dma_start(
            x_t[:], x_ap[tb : tb + CHUNK, :].rearrange("(t p) d -> p t d", p=128)
        )

        # ---- transpose to xT: 4 tiles [128 dim, CHUNK tok] ----
        xts = []
        for db in range(4):
            xtp = pmm.tile([128, CHUNK], F32, tag="mm")
            for t in range(SPC):
                nc.tensor.transpose(
                    xtp[:, ts(t, 128)], x_t[:, t, ts(db, 128)], ident[:]
                )
            xt = pxt.tile([128, CHUNK], mm_dt, tag="xt")
            nc.any.tensor_copy(xt[:], xtp[:])
            xts.append(xt)

        # ---- qT, kT in [feat, tok] layout: 8 ptiles of 128 feats ----
        qkts = []
        for p in range(8):
            ps = pmm.tile([128, CHUNK], F32, tag="mm")
            for kt in range(4):
                nc.tensor.matmul(
                    ps[:],
                    w_kts[kt][:, ts(p, 128)],
                    xts[kt][:],
                    start=(kt == 0),
                    stop=(kt == 3),
                )
            qs = pqk.tile([128, CHUNK], mm_dt, tag="qk", bufs=10)
            nc.vector.tensor_copy(qs[:], ps[:])
            qkts.append(qs)
        # odd heads live at partitions 64-127; matmul operands must sit at
        # base partition 0 (tile_position row 64 faults on this runtime), so
        # shift them down with SBUF->SBUF DMA (DMA is address-based).
        qkos = []
        for p in range(8):
            qo = pqk.tile([64, CHUNK], mm_dt, tag="qko", name=f"qko{ci}_{p}", bufs=9)
            nc.sync.dma_start(qo[:], qkts[p][64:128, :])
            qkos.append(qo)

        # ---- v natural [tok, feat] + ones column -> vext [128, h, 65] ----
        vexts = []
        for t in range(SPC):
            ps = pmm.tile([128, INNER], F32, tag="mm")
            for kt in range(4):
                nc.tensor.matmul(
                    ps[:],
                    xts[kt][:, ts(t, 128)],
                    w_kts[kt][:, 2 * INNER : 3 * INNER],
                    start=(kt == 0),
                    stop=(kt == 3),
                )
            vx = pvx.tile([128, HEADS, DH + 1], mm_dt, tag="vx", bufs=5)
            nc.vector.memset(vx[:, :, DH : DH + 1].bitcast(F32), 1.0)
            nc.vector.tensor_copy(
                vx[:, :, 0:DH], ps[:].rearrange("p (h d) -> p h d", h=HEADS)
            )
            vexts.append(vx)

        return qkts, qkos, vexts

    def attn(ci, st):
        qkts, qkos, vexts = st
        # ---- attention, output written transposed into outT ptiles ----
        # Even heads (rows 0-63 of a ptile) write otls directly; odd heads
        # compute into base-0 tiles (oto) and are DMA-shifted to rows 64-127.
        otls = [
            pot.tile([128, CHUNK], mm_dt, tag="ot", name=f"ot{ci}_{i}")
            for i in range(4)
        ]
        otos = [
            pot.tile([64, CHUNK], mm_dt, tag="oto", name=f"oto{ci}_{i}")
            for i in range(4)
        ]
        for fi in range(FPC):
            f0 = fi * FRAME
            for q in range(2):  # head quads
                pts = []
                for jt in range(2):  # key-side 128-token tiles of the frame
                    sim = psim.tile([128, 4 * FRAME], F32, tag="sim")
                    for hh in range(4):
                        h = q * 4 + hh
                        if h % 2 == 0:
                            ck = qkts[4 + h // 2][0:64, :]
                            cq = qkts[h // 2][0:64, :]
                        else:
                            ck = qkos[4 + h // 2][:]
                            cq = qkos[h // 2][:]
                        nc.tensor.matmul(
                            sim[:, ts(hh, FRAME)],
                            ck[:, f0 + jt * 128 : f0 + (jt + 1) * 128],
                            cq[:, f0 : f0 + FRAME],
                            start=True,
                            stop=True,
                        )
                    pt = ppt.tile([128, 4 * FRAME], mm_dt, tag="pt")
                    nc.scalar.activation(pt[:], sim[:], AF.Exp, scale=SCALE)
                    pts.append(pt)
                for m2 in range(2):  # head pairs within the quad
                    ovp = povp.tile([DH + 1, 2 * FRAME], F32, tag="ovp")
                    for hp in range(2):
                        h = q * 4 + 2 * m2 + hp
                        hh = 2 * m2 + hp
                        for jt in range(2):
                            nc.tensor.matmul(
                                ovp[:, ts(hp, FRAME)],
                                vexts[fi * 2 + jt][:, h, :],
                                pts[jt][:, ts(hh, FRAME)],
                                start=(jt == 0),
                                stop=(jt == 1),
                            )
                    # softmax denominators for both heads: rz = exp(-ln Z)
                    lnz = prz.tile([1, 2 * FRAME], F32, tag="lnz")
                    nc.scalar.activation(lnz[:], ovp[DH : DH + 1, :], AF.Ln)
                    # unnormalized outputs to SBUF; frees the PSUM tile
                    ovs = pov.tile([DH, 2 * FRAME], F32, tag="ovs")
                    nc.vector.tensor_copy(ovs[:], ovp[0:DH, :])
                    rz = prz.tile([1, 2 * FRAME], F32, tag="rz")
                    nc.scalar.activation(rz[:], lnz[:], AF.Exp, scale=-1.0)
                    rb = prb.tile([DH, 2 * FRAME], F32, tag="rb")
                    nc.gpsimd.partition_broadcast(rb[:], rz[:])
                    for hp in range(2):
                        h = q * 4 + 2 * m2 + hp
                        dst = otls[h // 2][0:DH] if h % 2 == 0 else otos[h // 2][:]
                        nc.vector.tensor_mul(
                            dst[:, f0 : f0 + FRAME],
                            ovs[:, ts(hp, FRAME)],
                            rb[:, ts(hp, FRAME)],
                        )
                    if fi == FPC - 1:
                        # odd head of this pair is complete: shift its rows
                        # into the ptile now so proj isn't gated on one big
                        # end-of-chunk DMA
                        p = q * 2 + m2
                        nc.sync.dma_start(otls[p][64:128, :], otos[p][:])

        return otls

    def proj(ci, otls):
        tb = ci * CHUNK
        # ---- output projection + bias ----
        for s in range(SPC):
            ps = pmm.tile([128, D], F32, tag="mm")
            for p in range(4):
                nc.tensor.matmul(
                    ps[:],
                    otls[p][:, ts(s, 128)],
                    wo_sb[:, p, :],
                    start=(p == 0),
                    stop=(p == 3),
                )
            y = py.tile([128, D], F32, tag="y", bufs=3)
            nc.vector.scalar_tensor_tensor(
                y[:], ps[:], 1.0, bb[:], op0=ALU.mult, op1=ALU.add
            )
            nc.sync.dma_start(out_ap[tb + s * 128 : tb + (s + 1) * 128, :], y[:])

    # Software pipeline: emit next chunk's front between this chunk's
    # attention and projection so PE has work while the softmax-normalize
    # chain (ACT->DVE->GPSIMD->DVE) drains.
    st = front(0)
    for ci in range(n_chunks):
        otls = attn(ci, st)
        if ci + 1 < n_chunks:
            st = front(ci + 1)
        proj(ci, otls)


_CACHE = {}


def _get_nc(n_chunks=NCH):
    key = ("nc", n_chunks, str(MM_DT))
    if key in _CACHE:
        return _CACHE[key]
    from contextlib import ExitStack

    nc = bacc.Bacc("TRN2", target_bir_lowering=False, debug=False, num_devices=N_CORES)
    _pin_act_tables.arch = nc.m.arch
    _pin_act_tables()
    t_tok = n_chunks * CHUNK
    x_ap = nc.dram_tensor("x", [t_tok, D], F32, kind="ExternalInput").ap()
    wqkv_ap = nc.dram_tensor("w_qkv", [D, 3 * INNER], F32, kind="ExternalInput").ap()
    wout_ap = nc.dram_tensor("w_out", [INNER, D], F32, kind="ExternalInput").ap()
    bout_ap = nc.dram_tensor("b_out", [D], F32, kind="ExternalInput").ap()
    out_ap = nc.dram_tensor("out", [t_tok, D], F32, kind="ExternalOutput").ap()
    with tile.TileContext(nc) as tc:
        with ExitStack() as ctx:
            _build_body(
                nc, tc, ctx, x_ap, wqkv_ap, wout_ap, bout_ap, out_ap, n_chunks=n_chunks
            )
    nc.compile()
    _CACHE[key] = nc
    return nc


def _make_in_maps(x, w_qkv, w_out, b_out):
    x = np.ascontiguousarray(np.asarray(x, dtype=np.float32))
    w_qkv = np.ascontiguousarray(np.asarray(w_qkv, dtype=np.float32))
    w_out = np.ascontiguousarray(np.asarray(w_out, dtype=np.float32))
    b_out = np.ascontiguousarray(np.asarray(b_out, dtype=np.float32))
    assert x.shape == (B, SEQ, D), x.shape
    in_maps = []
    for c in range(N_CORES):
        b = c // 2
        t0 = (c % 2) * T
        in_maps.append(
            {
                "x": np.ascontiguousarray(x[b, t0 : t0 + T, :]),
                "w_qkv": w_qkv,
                "w_out": w_out,
                "b_out": b_out,
            }
        )
    return in_maps


def _assemble(results):
    out = np.empty((B, SEQ, D), dtype=np.float32)
    for c in range(N_CORES):
        b = c // 2
        t0 = (c % 2) * T
        out[b, t0 : t0 + T, :] = results[c]["out"]
    return out


def run(x, w_qkv, w_out, b_out, f=32, trace=False):
    assert int(f) == 32, f"kernel hardcoded for f=32, got {f}"
    _install_ntff_hook()
    nc = _get_nc()
    in_maps = _make_in_maps(x, w_qkv, w_out, b_out)
    res = run_bass_kernel_spmd(nc, in_maps, list(range(N_CORES)), trace=trace)
    return _assemble(res.results), res


def kernel(x, w_qkv, w_out, b_out, f=32):
    out, _ = run(x, w_qkv, w_out, b_out, f=f, trace=False)
    return out


# revision 37
# speedup vs baseline: 4.0046x; 1.0199x over previous
"""Axial (frame-local) attention kernel for Trainium2, 8-core data-parallel.

Problem: x[4, 8192, 512] -> qkv proj -> per-(batch, head, frame) attention over
256-token frames (f=32 frames of 256 tokens in an 8192 sequence) -> out proj.

Sharding: pure data-parallel over (batch, half-sequence): core c handles
batch c//2, tokens (c%2)*4096 .. +4096 (16 whole frames). No collectives.

Per-core pipeline (chunks of 512 tokens):
  - load x chunk, PE-transpose into xT [dim, tok] (feature-major)
  - qT,kT = (w_qkv block)^T-matmul in [feat, tok] layout; v natural [tok, feat]
  - per (frame, head): sim^T = k q^T on PE -> exp on ScalarE (no max-subtract;
    logits are O(6) so fp32 exp is safe) -> ov = [v|1]^T p~ on PE produces both
    the unnormalized attention output AND the softmax denominator Z (row 64)
  - normalize: 1/Z = exp(-ln Z) on ScalarE (DVE reciprocal is 8 cyc/elem —
    too slow), GPSIMD partition-broadcast, one DVE multiply
  - output projection from the transposed layout + bias, DMA out

Matmul operands use float32r (single-pass fp32, ~tf32 precision, 2x faster
than fp32's LOW_HIGH two-pass mode). PSUM accumulation stays fp32.
"""

import sys
import types

import numpy as np

import concourse.bass as bass
import concourse.tile as tile
from concourse import bacc, mybir
from concourse.bass import ts
from concourse.bass_utils import run_bass_kernel_spmd
from concourse.masks import make_identity

F32 = mybir.dt.float32
F32R = mybir.dt.float32r
AF = mybir.ActivationFunctionType
ALU = mybir.AluOpType

# Model dims (hardcoded per problem spec)
B, SEQ, D = 4, 8192, 512
HEADS, DH = 8, 64
INNER = HEADS * DH  # 512
SCALE = DH ** -0.5
FRAME = 256  # n_sp = seq // f = 8192 // 32
N_CORES = 8
T = (B * SEQ) // N_CORES  # 4096 tokens per core
CHUNK = 512  # tokens per inner iteration
NCH = T // CHUNK  # 8
FPC = CHUNK // FRAME  # frames per chunk = 2
SPC = CHUNK // 128  # 128-token subtiles per chunk = 4

# matmul operand dtype: F32R (single-pass, ~tf32) or F32 (two-pass, exact)
MM_DT = F32R

FEATURES = set()  # retained for debug scripts


def _install_ntff_hook():
    """The trimmed container's antenv lacks axon_hooks; inject it so
    run_bass_kernel_spmd(trace=True) can capture NTFF profiles."""
    if "antenv.axon_hooks" in sys.modules:
        return
    try:
        from trn_agent_boot.trn_boot import _ntff_profile_via_ctypes

        hook = _ntff_profile_via_ctypes("/opt/axon/libaxon_pjrt.so")
    except Exception:
        return
    mod = types.ModuleType("antenv.axon_hooks")
    mod._hook = hook
    mod.get_axon_ntff_profile_hook = lambda: mod._hook
    mod.set_axon_ntff_profile_hook = lambda h: setattr(mod, "_hook", h)
    sys.modules["antenv.axon_hooks"] = mod


def _pin_act_tables():
    """Exp and Ln both live in the natural_log_exp_and_others table set, but
    the table-load chooser maps each function to the first set containing it,
    so alternating Exp/Ln activations reload tables (~1.3us each) every head
    pair. Restrict Exp/Ln to the combined set in the cached table map so one
    load covers the whole kernel."""
    from concourse.hw_specs import get_activation_tables

    tabs = get_activation_tables(_pin_act_tables.arch)
    keep = "natural_log_exp_and_others"
    if keep not in tabs:
        return
    for name, fns in tabs.items():
        if name != keep:
            fns.discard(AF.Exp)
            fns.discard(AF.Ln)


def _build_body(nc, tc, ctx, x_ap, wqkv_ap, wout_ap, bout_ap, out_ap, n_chunks=NCH):
    mm_dt = MM_DT
    pconst = ctx.enter_context(tc.tile_pool(name="const", bufs=1))
    px = ctx.enter_context(tc.tile_pool(name="x", bufs=2))
    pxt = ctx.enter_context(tc.tile_pool(name="xt", bufs=8))
    pqk = ctx.enter_context(tc.tile_pool(name="qk", bufs=16))
    pvx = ctx.enter_context(tc.tile_pool(name="vx", bufs=6))
    ppt = ctx.enter_context(tc.tile_pool(name="pt", bufs=4))
    prz = ctx.enter_context(tc.tile_pool(name="rz", bufs=3))
    prb = ctx.enter_context(tc.tile_pool(name="rb", bufs=3))
    pov = ctx.enter_context(tc.tile_pool(name="ovs", bufs=3))
    pot = ctx.enter_context(tc.tile_pool(name="ot", bufs=6))
    py = ctx.enter_context(tc.tile_pool(name="y", bufs=3))
    pmm = ctx.enter_context(tc.tile_pool(name="mm", bufs=2, space="PSUM"))
    psim = ctx.enter_context(tc.tile_pool(name="sim", bufs=2, space="PSUM"))
    povp = ctx.enter_context(tc.tile_pool(name="ovp", bufs=2, space="PSUM"))

    # Constants
    ident = pconst.tile([128, 128], F32, tag="ident")
    make_identity(nc, ident[:])
    w_kts = []
    for kt in range(4):
        wk = pconst.tile([128, 3 * INNER], mm_dt, tag=f"wqkv{kt}", name=f"wqkv{kt}")
        nc.scalar.dma_start(
            wk[:], wqkv_ap.bitcast(mm_dt)[kt * 128 : (kt + 1) * 128, :]
        )
        w_kts.append(wk)
    wo_sb = pconst.tile([128, 4, D], mm_dt, tag="wout")
    nc.scalar.dma_start(
        wo_sb[:], wout_ap.bitcast(mm_dt).rearrange("(kt p) e -> p kt e", p=128)
    )
    b1 = pconst.tile([1, D], F32, tag="b1")
    nc.scalar.dma_start(b1[:], bout_ap.rearrange("(a d) -> a d", a=1))
    bb = pconst.tile([128, D], F32, tag="bb")
    nc.gpsimd.partition_broadcast(bb[:], b1[:])

    def front(ci):
        tb = ci * CHUNK

        # ---- load x chunk [128, subtile, D] (token-major) ----
        x_t = px.tile([128, SPC, D], F32, tag="x")
        nc.sync.dma_start(
            x_t[:], x_ap[tb : tb + CHUNK, :].rearrange("(t p) d -> p t d", p=128)
        )

        # ---- transpose to xT: 4 tiles [128 dim, CHUNK tok] ----
        xts = []
        for db in range(4):
            xtp = pmm.tile([128, CHUNK], F32, tag="mm")
            for t in range(SPC):
                nc.tensor.transpose(
                    xtp[:, ts(t, 128)], x_t[:, t, ts(db, 128)], ident[:]
                )
            xt = pxt.tile([128, CHUNK], mm_dt, tag="xt")
            nc.any.tensor_copy(xt[:], xtp[:])
            xts.append(xt)

        # ---- qT, kT in [feat, tok] layout: 8 ptiles of 128 feats ----
        qkts = []
        qkos = []
        for p in range(8):
            ps = pmm.tile([128, CHUNK], F32, tag="mm")
            for kt in range(4):
                nc.tensor.matmul(
                    ps[:],
                    w_kts[kt][:, ts(p, 128)],
                    xts[kt][:],
                    start=(kt == 0),
                    stop=(kt == 3),
                )
            qs = pqk.tile([128, CHUNK], mm_dt, tag="qk", bufs=10)
            nc.vector.tensor_copy(qs[:], ps[:])
            qkts.append(qs)
            # odd heads live at partitions 64-127; matmul operands must sit
            # at base partition 0 (tile_position row 64 faults on this
            # runtime), so shift them down with SBUF->SBUF DMA right after
            # the cast (DMA is address-based)
            qo = pqk.tile([64, CHUNK], mm_dt, tag="qko", name=f"qko{ci}_{p}", bufs=9)
            nc.sync.dma_start(qo[:], qs[64:128, :])
            qkos.append(qo)

        # ---- v natural [tok, feat] + ones column -> vext [128, h, 65] ----
        vexts = []
        for t in range(SPC):
            ps = pmm.tile([128, INNER], F32, tag="mm")
            for kt in range(4):
                nc.tensor.matmul(
                    ps[:],
                    xts[kt][:, ts(t, 128)],
                    w_kts[kt][:, 2 * INNER : 3 * INNER],
                    start=(kt == 0),
                    stop=(kt == 3),
                )
            vx = pvx.tile([128, HEADS, DH + 1], mm_dt, tag="vx", bufs=5)
            nc.vector.memset(vx[:, :, DH : DH + 1].bitcast(F32), 1.0)
            nc.vector.tensor_copy(
                vx[:, :, 0:DH], ps[:].rearrange("p (h d) -> p h d", h=HEADS)
            )
            vexts.append(vx)

        return qkts, qkos, vexts

    def attn(ci, st):
        qkts, qkos, vexts = st
        # ---- attention, output written transposed into outT ptiles ----
        # Even heads (rows 0-63 of a ptile) write otls directly; odd heads
        # compute into base-0 tiles (oto) and are DMA-shifted to rows 64-127.
        otls = [
            pot.tile([128, CHUNK], mm_dt, tag="ot", name=f"ot{ci}_{i}")
            for i in range(4)
        ]
        otos = [
            pot.tile([64, CHUNK], mm_dt, tag="oto", name=f"oto{ci}_{i}")
            for i in range(4)
        ]
        for fi in range(FPC):
            f0 = fi * FRAME
            for q in range(2):  # head quads
                pts = []
                for jt in range(2):  # key-side 128-token tiles of the frame
                    sim = psim.tile([128, 4 * FRAME], F32, tag="sim")
                    for hh in range(4):
                        h = q * 4 + hh
                        if h % 2 == 0:
                            ck = qkts[4 + h // 2][0:64, :]
                            cq = qkts[h // 2][0:64, :]
                        else:
                            ck = qkos[4 + h // 2][:]
                            cq = qkos[h // 2][:]
                        nc.tensor.matmul(
                            sim[:, ts(hh, FRAME)],
                            ck[:, f0 + jt * 128 : f0 + (jt + 1) * 128],
                            cq[:, f0 : f0 + FRAME],
                            start=True,
                            stop=True,
                        )
                    pt = ppt.tile([128, 4 * FRAME], mm_dt, tag="pt")
                    nc.scalar.activation(pt[:], sim[:], AF.Exp, scale=SCALE)
                    pts.append(pt)
                for m2 in range(2):  # head pairs within the quad
                    ovp = povp.tile([DH + 1, 2 * FRAME], F32, tag="ovp")
                    for hp in range(2):
                        h = q * 4 + 2 * m2 + hp
                        hh = 2 * m2 + hp
                        for jt in range(2):
                            nc.tensor.matmul(
                                ovp[:, ts(hp, FRAME)],
                                vexts[fi * 2 + jt][:, h, :],
                                pts[jt][:, ts(hh, FRAME)],
                                start=(jt == 0),
                                stop=(jt == 1),
                            )
                    # softmax denominators for both heads: rz = exp(-ln Z)
                    lnz = prz.tile([1, 2 * FRAME], F32, tag="lnz")
                    nc.scalar.activation(lnz[:], ovp[DH : DH + 1, :], AF.Ln)
                    # unnormalized outputs to SBUF; frees the PSUM tile
                    ovs = pov.tile([DH, 2 * FRAME], F32, tag="ovs")
                    nc.vector.tensor_copy(ovs[:], ovp[0:DH, :])
                    rz = prz.tile([1, 2 * FRAME], F32, tag="rz")
                    nc.scalar.activation(rz[:], lnz[:], AF.Exp, scale=-1.0)
                    rb = prb.tile([DH, 2 * FRAME], F32, tag="rb")
                    nc.gpsimd.partition_broadcast(rb[:], rz[:])
                    for hp in range(2):
                        h = q * 4 + 2 * m2 + hp
                        dst = otls[h // 2][0:DH] if h % 2 == 0 else otos[h // 2][:]
                        nc.vector.tensor_mul(
                            dst[:, f0 : f0 + FRAME],
                            ovs[:, ts(hp, FRAME)],
                            rb[:, ts(hp, FRAME)],
                        )
                    if fi == FPC - 1:
                        # odd head of this pair is complete: shift its rows
                        # into the ptile now so proj isn't gated on one big
                        # end-of-chunk DMA
                        p = q * 2 + m2
                        nc.sync.dma_start(otls[p][64:128, :], otos[p][:])

        return otls

    def proj(ci, otls):
        tb = ci * CHUNK
        # ---- output projection + bias ----
        for s in range(SPC):
            ps = pmm.tile([128, D], F32, tag="mm")
            for p in range(4):
                nc.tensor.matmul(
                    ps[:],
                    otls[p][:, ts(s, 128)],
                    wo_sb[:, p, :],
                    start=(p == 0),
                    stop=(p == 3),
                )
            y = py.tile([128, D], F32, tag="y", bufs=3)
            nc.vector.scalar_tensor_tensor(
                y[:], ps[:], 1.0, bb[:], op0=ALU.mult, op1=ALU.add
            )
            nc.sync.dma_start(out_ap[tb + s * 128 : tb + (s + 1) * 128, :], y[:])

    # Software pipeline: emit next chunk's front between this chunk's
    # attention and projection so PE has work while the softmax-normalize
    # chain (ACT->DVE->GPSIMD->DVE) drains.
    st = front(0)
    for ci in range(n_chunks):
        otls = attn(ci, st)
        if ci + 1 < n_chunks:
            st = front(ci + 1)
        proj(ci, otls)


_CACHE = {}


def _get_nc(n_chunks=NCH):
    key = ("nc", n_chunks, str(MM_DT))
    if key in _CACHE:
        return _CACHE[key]
    from contextlib import ExitStack

    nc = bacc.Bacc("TRN2", target_bir_lowering=False, debug=False, num_devices=N_CORES)
    _pin_act_tables.arch = nc.m.arch
    _pin_act_tables()
    t_tok = n_chunks * CHUNK
    x_ap = nc.dram_tensor("x", [t_tok, D], F32, kind="ExternalInput").ap()
    wqkv_ap = nc.dram_tensor("w_qkv", [D, 3 * INNER], F32, kind="ExternalInput").ap()
    wout_ap = nc.dram_tensor("w_out", [INNER, D], F32, kind="ExternalInput").ap()
    bout_ap = nc.dram_tensor("b_out", [D], F32, kind="ExternalInput").ap()
    out_ap = nc.dram_tensor("out", [t_tok, D], F32, kind="ExternalOutput").ap()
    with tile.TileContext(nc) as tc:
        with ExitStack() as ctx:
            _build_body(
                nc, tc, ctx, x_ap, wqkv_ap, wout_ap, bout_ap, out_ap, n_chunks=n_chunks
            )
    nc.compile()
    _CACHE[key] = nc
    return nc


def _make_in_maps(x, w_qkv, w_out, b_out):
    x = np.ascontiguousarray(np.asarray(x, dtype=np.float32))
    w_qkv = np.ascontiguousarray(np.asarray(w_qkv, dtype=np.float32))
    w_out = np.ascontiguousarray(np.asarray(w_out, dtype=np.float32))
    b_out = np.ascontiguousarray(np.asarray(b_out, dtype=np.float32))
    assert x.shape == (B, SEQ, D), x.shape
    in_maps = []
    for c in range(N_CORES):
        b = c // 2
        t0 = (c % 2) * T
        in_maps.append(
            {
                "x": np.ascontiguousarray(x[b, t0 : t0 + T, :]),
                "w_qkv": w_qkv,
                "w_out": w_out,
                "b_out": b_out,
            }
        )
    return in_maps


def _assemble(results):
    out = np.empty((B, SEQ, D), dtype=np.float32)
    for c in range(N_CORES):
        b = c // 2
        t0 = (c % 2) * T
        out[b, t0 : t0 + T, :] = results[c]["out"]
    return out


def run(x, w_qkv, w_out, b_out, f=32, trace=False):
    assert int(f) == 32, f"kernel hardcoded for f=32, got {f}"
    _install_ntff_hook()
    nc = _get_nc()
    in_maps = _make_in_maps(x, w_qkv, w_out, b_out)
    res = run_bass_kernel_spmd(nc, in_maps, list(range(N_CORES)), trace=trace)
    return _assemble(res.results), res


def kernel(x, w_qkv, w_out, b_out, f=32):
    out, _ = run(x, w_qkv, w_out, b_out, f=f, trace=False)
    return out
